# revision 1
# baseline (speedup 1.0000x reference)
"""Trainium2 Bass kernel for nn_Conv3DNorm (modulated conv3d + demod + lrelu + clamp).

Reference math (styles == ones):
    dcoef[cout] = rsqrt(sum_{cin,kd,kh,kw} weight^2 + 1e-8)
    y = conv3d(x, weight * dcoef, pad=1)            # per-sample, stride 1
    y = leaky_relu(y + bias, 0.2) * sqrt(2)
    y = clip(y, -256, 256)

Sharding: data-parallel over batch. Core i processes sample i (B=8 == n_cores).
Weight/bias replicated. Host prep is layout/dtype only (transpose, pad, cast).

Design notes (from HW traces):
  - conv = 27 accumulated bf16 matmuls per 512-position output chunk
    (chunk = (depth, half-of-H)); 64 chunks; PSUM bank per chunk, 7-bank
    rotation.  bf16 N=512 matmuls run at the roofline (213.3 + ~2.5 ns
    issue = 216 ns); f32r pays +30 ns/MM.  rel err ~2e-3 (gate 2e-2).
  - matmuls whose rhs base is 2-byte-misaligned (odd element) pay +13 ns.
    kw=1 taps never touch the W halo, so they read from `xpad2` (x with
    H-padding only, width 32: 4B-aligned rows); kw=0/2 read the fully
    padded `xpad`.  All bases even -> no penalty anywhere.
  - x is PRE-PADDED ON HOST in both layouts, so every DMA is a contiguous
    line-rate transfer.  On-chip padding was tried twice and lost: DMA
    into a padded layout degenerates to 64B-run descriptors (~20x slow),
    and DVE halo-column memsets are scattered 2B writes costing ~4.4us
    each (read-modify-write), stalling the startup-critical DVE queue.
  - HAM warm-up: the PE runs at 1.2 GHz until ~3.4us of sustained
    activity; throwaway matmuls on memset tiles (ready ~7.5us) bring it
    to 2.4 GHz just as the first real matmul's data lands (~10.5us).
  - dcoef: squares+reduce on DVE (2 ops), one PE matmul with a ones
    vector emitted after chunk 4 (so the in-order PE queue reaches it
    only after the DVE chain finished), then sqrt/reciprocal; the
    epilogue folds dcoef*sqrt2 and bias*sqrt2.
  - epilogue per chunk is 3 DVE ops: u = ps*(sqrt2*dcoef) + sqrt2*bias;
    v = max(0.2*u, u)  (== sqrt2*leaky_relu);  out = clip(v, +-256).
"""

import os
import sys

for _p in (
    "/root/.axon_site",
    "/root/.axon_site/_ro/trn_rl_repo",
    "/root/.axon_site/_ro/pypackages",
):
    if os.path.isdir(_p) and _p not in sys.path:
        sys.path.insert(0, _p)

import numpy as np

import concourse.bass as bass  # noqa: F401
import concourse.mybir as mybir
import concourse.tile as tile
from concourse import bacc
from concourse.bass_utils import run_bass_kernel_spmd

# Problem constants (hardcoded per contract).
B = 8
CIN = 128
COUT = 128
D = H = W = 32
K = 3
NTAPS = K * K * K  # 27
HP = H + 2  # 34
WP = W + 2  # 34
NCHUNK = 64  # output chunks of 512 spatial positions: (d, half-of-H)
EPS = 1e-8
S1 = float(np.sqrt(2.0))  # ACT_GAIN * GAIN
CLAMP = 256.0
ALPHA = 0.2
NWARM = 12  # HAM warm-up matmuls: span past the first real matmul's data
            # arrival (~12.5us, HBM-contention-bound) so the PE ramp never
            # pauses -- a gap resets the 3.4us sustained-activity window.
DC_CHUNK = 3  # emit the dcoef matmul after this chunk: late enough that the
              # in-order PE queue reaches it after the DVE square/tree-sum
              # finished, early enough that the deferred epilogues free their
              # PSUM banks before the 7-bank rotation needs them (chunk 7).

# matmul dtype: "bf16" (roofline) or "f32r" (TF32-like, +30ns/MM)
MM_MODE = os.environ.get("CONV_MM_MODE", "bf16")

LAST_RESULTS = None  # BassKernelResults of the most recent run (for test.py)

_CACHED = {}


def _build_nc(mode: str):
    dt = mybir.dt
    io_dt = {"f32r": dt.float32r, "bf16": dt.bfloat16, "f32": dt.float32}[mode]

    nc = bacc.Bacc("TRN2")
    xp_d = nc.dram_tensor("xp", [CIN, D, HP, WP], io_dt, kind="ExternalInput")
    xp2_d = nc.dram_tensor("xp2", [CIN, D, HP, W], io_dt, kind="ExternalInput")
    w_d = nc.dram_tensor("w", [CIN, NTAPS, COUT], io_dt, kind="ExternalInput")
    b_d = nc.dram_tensor("bias", [COUT, 1], dt.float32, kind="ExternalInput")
    # output stays fp32: switching the oc tiles to bf16 shifted the SBUF
    # allocation map and slowed EVERY matmul by ~50ns (placement-sensitive
    # bank conflicts between the epilogue tiles and the PE-streamed tiles).
    y_d = nc.dram_tensor("y", [COUT, NCHUNK, 512], dt.float32, kind="ExternalOutput")

    def asf32(ap):
        return ap.bitcast(dt.float32) if mode == "f32r" else ap

    with tile.TileContext(nc) as tc:
        with (
            tc.tile_pool(name="big", bufs=1) as big,
            tc.tile_pool(name="small", bufs=1) as small,
            tc.tile_pool(name="sq", bufs=1) as sqp,
            tc.tile_pool(name="epiv", bufs=4) as vp,
            tc.tile_pool(name="epio", bufs=4) as op,
        ):
            # xpad2 allocated FIRST: with it placed after xpad, the kw=1
            # matmuls streaming from it measured +44ns each (SBUF placement
            # effect, reproducible); this order shows none.
            xpad2 = big.tile([CIN, D, HP, W], io_dt)   # H-padded only
            xpad = big.tile([CIN, D, HP, WP], io_dt)   # (H,W)-padded
            w_sb = big.tile([CIN, NTAPS, COUT], io_dt)
            bias_sb = small.tile([COUT, 1], dt.float32)

            # warm-up operands (memset, ready ~7.5us -- before any DMA lands)
            warm_w = small.tile([CIN, COUT], io_dt)
            nc.vector.memset(asf32(warm_w[:]), 0.0)
            warm_x = small.tile([CIN, 512], io_dt)
            nc.vector.memset(asf32(warm_x[:]), 0.0)

            # ---- DMAs, ordered for chunk 0 (which runs taps 9..26 first,
            # reading slices d0/d1); both HWDGE rings used, all contiguous.
            def wpiece(g):
                nc.sync.dma_start(
                    w_sb[:, 9 * g : 9 * (g + 1), :], w_d[:, 9 * g : 9 * (g + 1), :]
                )

            def xslice(d):
                nc.sync.dma_start(xpad[:, d], xp_d[:, d])
                nc.scalar.dma_start(xpad2[:, d], xp2_d[:, d])

            # Only slices d0-d6 are issued up front: the scalar-ring DMAs are
            # issued by the ACT sequencer, and anything queued there delays
            # later ACT work; the bulk is emitted after the dcoef block.
            # Startup is HBM-contention-bound (8 cores load simultaneously),
            # so splitting these transfers finer just moves the stalls.
            wpiece(1)                       # taps 9-17 (chunk 0's first taps)
            nc.scalar.dma_start(bias_sb[:], b_d[:])
            xslice(0)
            xslice(1)
            wpiece(2)                       # taps 18-26
            xslice(2)
            wpiece(0)                       # taps 0-8 (first used by chunk 2)
            for d in range(3, 7):
                xslice(d)

            # ---- demodulation coefficients ----
            scal = {}

            def emit_dcoef(warm_ps):
                # acc[cin,cout] = sum_tap w^2 (2 DVE ops), then one matmul
                # with ones reduces over cin: ps_dc[cout,1] = acc.T @ ones.
                ones = small.tile([CIN, 1], dt.float32)
                nc.vector.memset(ones[:], 1.0)
                wflat = asf32(w_sb[:]).rearrange("c t f -> c (t f)")
                sq = sqp.tile([CIN, NTAPS, COUT], dt.float32)
                sqf = sq[:].rearrange("c t f -> c (t f)")
                nc.vector.tensor_mul(sqf, wflat, wflat)
                # tap-reduction as a contiguous tree (a strided tensor_reduce
                # pays the scattered-access penalty, ~4x slower)
                n = NTAPS
                while n > 1:
                    k = n // 2
                    nc.vector.tensor_add(
                        sq[:, 0:k, :], sq[:, 0:k, :], sq[:, n - k : n, :]
                    )
                    n -= k
                acc = sq[:, 0, :]
                ps_dc = warm_ps[:, 0:1]
                nc.tensor.matmul(ps_dc, acc, ones[:], start=True, stop=True)
                # dscale = sqrt(2) * rsqrt(sums + eps), computed ENTIRELY on
                # DVE (bit-hack seed + 2 Newton steps, rel err ~1e-6).  The
                # ACT-engine Sqrt was reordered by the scheduler behind ~25
                # queued DMA issues, stalling the first epilogues ~20us.
                xe = small.tile([COUT, 1], dt.float32)
                nc.vector.tensor_scalar(
                    out=xe[:], in0=ps_dc, scalar1=float(EPS), scalar2=None,
                    op0=mybir.AluOpType.add,
                )
                r = small.tile([COUT, 1], dt.float32)
                # r0 = bitcast(0x5f3759df - (bitcast_int(x) >> 1))
                nc.vector.tensor_scalar(
                    out=r[:].bitcast(dt.int32),
                    in0=xe[:].bitcast(dt.int32),
                    scalar1=1,
                    scalar2=None,
                    op0=mybir.AluOpType.logical_shift_right,
                )
                nc.vector.tensor_scalar(
                    out=r[:].bitcast(dt.int32),
                    in0=r[:].bitcast(dt.int32),
                    scalar1=-1,
                    scalar2=0x5F3759DF,
                    op0=mybir.AluOpType.mult,
                    op1=mybir.AluOpType.add,
                )
                t1 = small.tile([COUT, 1], dt.float32)
                t2 = small.tile([COUT, 1], dt.float32)
                for _ in range(2):  # r <- r * (1.5 - 0.5 * x * r^2)
                    nc.vector.tensor_mul(t1[:], r[:], r[:])
                    nc.vector.tensor_mul(t2[:], t1[:], xe[:])
                    nc.vector.tensor_scalar(
                        out=t2[:], in0=t2[:], scalar1=-0.5, scalar2=1.5,
                        op0=mybir.AluOpType.mult, op1=mybir.AluOpType.add,
                    )
                    nc.vector.tensor_mul(r[:], r[:], t2[:])
                dscale = small.tile([COUT, 1], dt.float32)
                nc.vector.tensor_scalar_mul(dscale[:], r[:], S1)
                bias_s = small.tile([COUT, 1], dt.float32)
                nc.vector.tensor_scalar_mul(bias_s[:], bias_sb[:], S1)
                scal["dscale"] = dscale
                scal["bias_s"] = bias_s

            # ---- main conv loop (chunk-major: each chunk's 27 matmuls are
            # consecutive; chunk completions stagger and the epilogues
            # overlap the matmul stream; PE stays at HAM K=8/8 throughout) ----
            with (
                tc.tile_pool(name="ps", bufs=7, space="PSUM") as psp,
                tc.tile_pool(name="dcps", bufs=1, space="PSUM") as dcps,
            ):
                warm_ps = dcps.tile([COUT, 512], dt.float32, name="dc")
                for _ in range(NWARM):
                    nc.tensor.matmul(
                        warm_ps[:], warm_w[:], warm_x[:], start=True, stop=True
                    )

                def epilogue(c, ps, split=1):
                    # split=2 only for the last chunk: the final DMA's HBM
                    # receipt is on the critical path to kernel end, so the
                    # first half's store overlaps the second half's compute.
                    u = vp.tile([COUT, 512], dt.float32, name=f"u_{c}", tag="u")
                    v = vp.tile([COUT, 512], dt.float32, name=f"v_{c}", tag="v")
                    oc = op.tile([COUT, 512], dt.float32, name=f"oc_{c}", tag="oc")
                    n = 512 // split
                    for s in range(split):
                        sl = slice(s * n, (s + 1) * n)
                        nc.vector.tensor_scalar(
                            out=u[:, sl],
                            in0=ps[:, sl],
                            scalar1=scal["dscale"][:],
                            scalar2=scal["bias_s"][:],
                            op0=mybir.AluOpType.mult,
                            op1=mybir.AluOpType.add,
                        )
                        nc.vector.scalar_tensor_tensor(
                            out=v[:, sl],
                            in0=u[:, sl],
                            scalar=ALPHA,
                            in1=u[:, sl],
                            op0=mybir.AluOpType.mult,
                            op1=mybir.AluOpType.max,
                        )
                        nc.vector.tensor_scalar(
                            out=oc[:, sl],
                            in0=v[:, sl],
                            scalar1=-CLAMP,
                            scalar2=CLAMP,
                            op0=mybir.AluOpType.max,
                            op1=mybir.AluOpType.min,
                        )
                        nc.sync.dma_start(y_d[:, c, sl], oc[:, sl])

                def emit_mms(c, ps, taps, valid):
                    d, h0 = c // 2, (c % 2) * 16
                    for t in taps:
                        kd, kh, kw = t // 9, (t // 3) % 3, t % 3
                        if kw == 1:
                            rhs = xpad2[:, d + kd - 1, h0 + kh : h0 + kh + 16, :]
                        else:
                            rhs = xpad[
                                :, d + kd - 1, h0 + kh : h0 + kh + 16, kw : kw + 32
                            ]
                        nc.tensor.matmul(
                            ps[:],
                            w_sb[:, t, :],
                            rhs,
                            start=(t == valid[0]),
                            stop=(t == valid[-1]),
                        )

                pending = []
                for c in range(NCHUNK):
                    if c == 1:
                        continue  # emitted together with chunk 0
                    d = c // 2
                    ps = psp.tile([COUT, 512], dt.float32, name=f"ps_{c}", tag="ps")
                    valid = [t for t in range(NTAPS) if 0 <= d + t // 9 - 1 < D]
                    if c == 0:
                        # interleave chunks 0 and 1 at the kd-group level so
                        # the taps reading slice d1 start ~2us later, hiding
                        # its (HBM-contention-bound) arrival.
                        ps1 = psp.tile([COUT, 512], dt.float32, name="ps_1", tag="ps")
                        emit_mms(0, ps, valid[:9], valid)
                        emit_mms(1, ps1, valid[:9], valid)
                        emit_mms(0, ps, valid[9:], valid)
                        emit_mms(1, ps1, valid[9:], valid)
                        pending.append((0, ps))
                        pending.append((1, ps1))
                        continue
                    emit_mms(c, ps, valid, valid)
                    if c == NCHUNK - 1:
                        epilogue(c, ps, split=2)
                        continue
                    if c < DC_CHUNK:
                        # deferred: these epilogues need dcoef, emitted after
                        # chunk DC_CHUNK so the in-order PE queue reaches the
                        # dcoef matmul only after the DVE chain finished.
                        pending.append((c, ps))
                        continue
                    if c == DC_CHUNK:
                        emit_dcoef(warm_ps)
                        for pc, pps in pending:
                            epilogue(pc, pps)
                        # bulk slices in groups of 4: each DMA-completion sem
                        # wait carried by a chunk-start matmul costs a ~432ns
                        # PE bubble, so fewer sems -> fewer bubbles.
                        for dd in range(7, D, 4):
                            de = min(dd + 4, D)
                            nc.sync.dma_start(xpad[:, dd:de], xp_d[:, dd:de])
                            nc.scalar.dma_start(xpad2[:, dd:de], xp2_d[:, dd:de])
                    epilogue(c, ps)
    nc.compile()
    return nc


def _get_nc(mode: str):
    if mode not in _CACHED:
        _CACHED[mode] = _build_nc(mode)
    return _CACHED[mode]


def kernel(x: np.ndarray, weight: np.ndarray, bias: np.ndarray) -> np.ndarray:
    global LAST_RESULTS
    mode = MM_MODE
    if mode == "bf16":
        import ml_dtypes

        io = ml_dtypes.bfloat16
    else:
        io = np.float32

    x = np.asarray(x)
    weight = np.asarray(weight, dtype=np.float32)
    bias = np.asarray(bias, dtype=np.float32)

    # [cout, cin, kd, kh, kw] -> [cin, (kd kh kw), cout]
    w_prep = np.ascontiguousarray(
        weight.transpose(1, 2, 3, 4, 0).reshape(CIN, NTAPS, COUT).astype(io)
    )
    b_prep = np.ascontiguousarray(bias.reshape(COUT, 1))

    xio = x.astype(io)
    in_maps = []
    for i in range(B):
        xp = np.zeros((CIN, D, HP, WP), dtype=io)
        xp[:, :, 1 : H + 1, 1 : W + 1] = xio[i]
        xp2 = np.zeros((CIN, D, HP, W), dtype=io)
        xp2[:, :, 1 : H + 1, :] = xio[i]
        in_maps.append({"xp": xp, "xp2": xp2, "w": w_prep, "bias": b_prep})

    nc = _get_nc(mode)
    trace = bool(int(os.environ.get("CONV_TRACE", "0")))
    res = run_bass_kernel_spmd(
        nc,
        in_maps,
        core_ids=list(range(B)),
        trace=trace,
    )
    LAST_RESULTS = res
    out = np.stack(
        [r["y"].reshape(COUT, D, H, W) for r in res.results], axis=0
    ).astype(np.float32)
    return out



# revision 14
# speedup vs baseline: 1.2739x; 1.2739x over previous
"""Trainium2 Bass kernel for nn_Conv3DNorm (modulated conv3d + demod + lrelu + clamp).

Reference math (styles == ones):
    dcoef[cout] = rsqrt(sum_{cin,kd,kh,kw} weight^2 + 1e-8)
    y = conv3d(x, weight * dcoef, pad=1)            # per-sample, stride 1
    y = leaky_relu(y + bias, 0.2) * sqrt(2)
    y = clip(y, -256, 256)

Sharding: data-parallel over batch. Core i processes sample i (B=8 == n_cores).

Algorithm: 1D Winograd F(2,3) along the DEPTH axis.  The direct method needs
27 matmuls per 512-position chunk (64 chunks, 1692 matmuls total after
boundary-tap skips) = 361 us of bf16 PE time at N=512 roofline (213.3 ns +
~2.5 ns issue).  Winograd-D computes output-slice PAIRS (2jd, 2jd+1) from 4
transformed components, eliminating the kd tap dimension:
    xt[0] = xp[2jd]   - xp[2jd+2]        (xp = D/H/W zero-padded input)
    xt[1] = xp[2jd+1] + xp[2jd+2]
    xt[2] = xp[2jd+2] - xp[2jd+1]
    xt[3] = xp[2jd+1] - xp[2jd+3]
    m[t]  = conv2d_{kh,kw}(wc[t], xt[t])   # 9 taps x 4 comps = 36 matmuls
    y[2jd]   = m[0] + 0.5*(m[1] + m[2])    # 1/2 from the G-transform is
    y[2jd+1] = 0.5*(m[1] - m[2]) - m[3]    # folded into the inverse
with weight comps wc = [w[kd=0], w0+w1+w2, w0-w1+w2, w[kd=2]] (the scaled-by-
half comps are stored UNSCALED to save precision and DVE ops).  36 matmuls
per TWO direct chunks vs 54 -> 1.5x less PE time (~254 us).  bf16-pipeline
rel err simulated at 3.45e-3 (gate 2e-2).

Design notes carried over from the direct-conv baseline (HW-trace-derived):
  - bf16 N=512 matmuls run at roofline; matmuls whose rhs base is 2-byte-
    misaligned (odd element, the kw=1 taps) pay +13 ns; a second shifted
    xtilde copy does not fit SBUF (xtilde is 148 KB/partition), so ~5 us of
    penalty is accepted.
  - x arrives HOST-PADDED in all three dims ([CIN,34,34,34] bf16) so every
    DMA is a contiguous line-rate slice transfer; D-transform runs on DVE
    (whole-slice contiguous adds, bf16 2x rate) from a 6-slice raw ring.
  - HAM warm-up: throwaway matmuls on memset tiles keep the PE ramping to
    2.4 GHz before the first real matmul's data lands.  Four extra N=256
    probe matmuls measure the short-matmul cost for a future F(4,3) variant.
  - dcoef is computed entirely on DVE from a [cout,27,cin]-transposed bf16
    weight copy (square, tree-reduce taps, free-dim reduce over cin, then
    bit-hack rsqrt + 2 Newton steps) -- no PE matmul, no PSUM bank, none of
    the baseline's deferred-epilogue ordering dance.
  - PSUM: each chunk accumulates m[0..3] in a [cout,4,512] tile = 4 banks;
    pool bufs=2 uses all 8 banks; the warm-up tile shares slot 0 (write-
    after-write with chunk 1, safe in the in-order PE queue).
  - epilogue per chunk is 10 DVE ops (inverse transform + scale/lrelu/clamp
    for the even and odd output slices); DVE total ~75 us vs PE ~254 us.
"""

import os
import sys

for _p in (
    "/root/.axon_site",
    "/root/.axon_site/_ro/trn_rl_repo",
    "/root/.axon_site/_ro/pypackages",
):
    if os.path.isdir(_p) and _p not in sys.path:
        sys.path.insert(0, _p)

import numpy as np

import concourse.bass as bass  # noqa: F401
import concourse.mybir as mybir
import concourse.tile as tile
from concourse import bacc
from concourse.bass_utils import run_bass_kernel_spmd

# Problem constants (hardcoded per contract).
B = 8
CIN = 128
COUT = 128
D = H = W = 32
K = 3
PD = D + 2   # 34 padded depth slices
HP = H + 2   # 34
WP = W + 2   # 34
NJD = 16     # depth output-slice pairs
NCHUNK = 32  # (jd, half-of-H) chunks; each yields 2x512 outputs
EPS = 1e-8
S1 = float(np.sqrt(2.0))  # ACT_GAIN * GAIN
CLAMP = 256.0
ALPHA = 0.2
NWARM = 12
NPROBE = 4   # N=256 probe matmuls (F(4,3) feasibility measurement)
RING = 5     # raw-slice ring depth

LAST_RESULTS = None  # BassKernelResults of the most recent run (for test.py)

_CACHED = {}


def _build_nc():
    dt = mybir.dt
    io_dt = dt.bfloat16

    nc = bacc.Bacc("TRN2")
    xp_d = nc.dram_tensor("xp", [CIN, PD, HP, WP], io_dt, kind="ExternalInput")
    w_d = nc.dram_tensor("w", [CIN, K, 9, COUT], io_dt, kind="ExternalInput")
    wt_d = nc.dram_tensor("wt", [COUT, 27, CIN], io_dt, kind="ExternalInput")
    b_d = nc.dram_tensor("bias", [COUT, 1], dt.float32, kind="ExternalInput")
    y_d = nc.dram_tensor("y", [COUT, 2 * D, 512], dt.float32, kind="ExternalOutput")

    with tile.TileContext(nc) as tc:
        with (
            tc.tile_pool(name="big", bufs=1) as big,
            tc.tile_pool(name="ring", bufs=RING) as ring,
            tc.tile_pool(name="small", bufs=1) as small,
            tc.tile_pool(name="epi", bufs=2) as ep,
            tc.tile_pool(name="oc", bufs=3) as op,
        ):
            xtilde = big.tile([CIN, 4, NJD, HP, WP], io_dt)
            w_sb = big.tile([CIN, K, 9, COUT], io_dt)
            wt12 = big.tile([CIN, 2, 9, COUT], io_dt)
            wT_sb = big.tile([COUT, 27, CIN], io_dt)
            bias_sb = small.tile([COUT, 1], dt.float32)

            # warm-up operands (memset, ready before any DMA lands)
            warm_w = small.tile([CIN, COUT], io_dt)
            nc.vector.memset(warm_w[:], 0.0)
            warm_x = small.tile([CIN, 512], io_dt)
            nc.vector.memset(warm_x[:], 0.0)

            # ---- upfront DMAs, ordered for the startup critical path ----
            raw = {}

            def dma_slice(p, eng):
                raw[p] = ring.tile([CIN, HP, WP], io_dt, name=f"p{p}", tag="p")
                eng.dma_start(raw[p][:], xp_d[:, p])

            nc.sync.dma_start(w_sb[:], w_d[:])
            nc.scalar.dma_start(bias_sb[:], b_d[:])
            for p in range(4):  # transform(0) inputs
                dma_slice(p, nc.sync if p % 2 == 0 else nc.scalar)
            dma_slice(4, nc.sync)
            dma_slice(5, nc.scalar)
            nc.scalar.dma_start(wT_sb[:], wt_d[:])

            # ---- weight comps on DVE: wt12[0]=w0+w1+w2, wt12[1]=w0-w1+w2 ----
            wt_tmp = ep.tile([CIN, 9, COUT], io_dt, name="wt_tmp", tag="pq")
            nc.vector.tensor_add(wt_tmp[:], w_sb[:, 0], w_sb[:, 2])
            nc.vector.tensor_add(wt12[:, 0], wt_tmp[:], w_sb[:, 1])
            nc.vector.tensor_sub(wt12[:, 1], wt_tmp[:], w_sb[:, 1])

            # ---- depth transform for one jd (4 whole-slice DVE ops) ----
            def emit_transform(jd):
                r0, r1, r2, r3 = (raw[2 * jd + i] for i in range(4))
                nc.vector.tensor_sub(xtilde[:, 0, jd], r0[:], r2[:])
                nc.vector.tensor_add(xtilde[:, 1, jd], r1[:], r2[:])
                nc.vector.tensor_sub(xtilde[:, 2, jd], r2[:], r1[:])
                nc.vector.tensor_sub(xtilde[:, 3, jd], r1[:], r3[:])

            emit_transform(0)
            emit_transform(1)

            # ---- dcoef: DVE-only (square, tap tree-reduce, cin reduce,
            # bit-hack rsqrt + 2 Newton steps; the ACT-engine Sqrt was found
            # to reorder badly in the baseline) ----
            scal = {}

            def emit_dcoef():
                sqp = small.tile([COUT, 9, CIN], io_dt)
                acc_a = small.tile([COUT, CIN], dt.float32)
                acc_b = small.tile([COUT, CIN], dt.float32)
                nc.vector.memset(acc_a[:], 0.0)
                accs = [acc_a, acc_b]
                for g in range(3):
                    nc.vector.tensor_mul(
                        sqp[:], wT_sb[:, 9 * g : 9 * (g + 1), :],
                        wT_sb[:, 9 * g : 9 * (g + 1), :],
                    )
                    n = 9
                    while n > 1:
                        k = n // 2
                        nc.vector.tensor_add(
                            sqp[:, 0:k, :], sqp[:, 0:k, :], sqp[:, n - k : n, :]
                        )
                        n -= k
                    nc.vector.tensor_add(
                        accs[(g + 1) % 2][:], accs[g % 2][:], sqp[:, 0, :]
                    )
                acc = accs[1][:]  # after g=2, result is in acc_b
                dsum = small.tile([COUT, 1], dt.float32)
                nc.vector.tensor_reduce(
                    dsum[:], acc, axis=mybir.AxisListType.X, op=mybir.AluOpType.add
                )
                xe = small.tile([COUT, 1], dt.float32)
                nc.vector.tensor_scalar(
                    out=xe[:], in0=dsum[:], scalar1=float(EPS), scalar2=None,
                    op0=mybir.AluOpType.add,
                )
                r = small.tile([COUT, 1], dt.float32)
                nc.vector.tensor_scalar(
                    out=r[:].bitcast(dt.int32),
                    in0=xe[:].bitcast(dt.int32),
                    scalar1=1,
                    scalar2=None,
                    op0=mybir.AluOpType.logical_shift_right,
                )
                nc.vector.tensor_scalar(
                    out=r[:].bitcast(dt.int32),
                    in0=r[:].bitcast(dt.int32),
                    scalar1=-1,
                    scalar2=0x5F3759DF,
                    op0=mybir.AluOpType.mult,
                    op1=mybir.AluOpType.add,
                )
                t1 = small.tile([COUT, 1], dt.float32)
                t2 = small.tile([COUT, 1], dt.float32)
                for _ in range(2):  # r <- r * (1.5 - 0.5 * x * r^2)
                    nc.vector.tensor_mul(t1[:], r[:], r[:])
                    nc.vector.tensor_mul(t2[:], t1[:], xe[:])
                    nc.vector.tensor_scalar(
                        out=t2[:], in0=t2[:], scalar1=-0.5, scalar2=1.5,
                        op0=mybir.AluOpType.mult, op1=mybir.AluOpType.add,
                    )
                    nc.vector.tensor_mul(r[:], r[:], t2[:])
                dscale = small.tile([COUT, 1], dt.float32)
                nc.vector.tensor_scalar_mul(dscale[:], r[:], S1)
                bias_s = small.tile([COUT, 1], dt.float32)
                nc.vector.tensor_scalar_mul(bias_s[:], bias_sb[:], S1)
                scal["dscale"] = dscale
                scal["bias_s"] = bias_s

            emit_dcoef()

            # ---- main loop ----
            with tc.tile_pool(name="ps", bufs=2, space="PSUM") as psp:
                warm_ps = psp.tile([COUT, 4, 512], dt.float32, name="warm", tag="ps")
                for _ in range(NWARM):
                    nc.tensor.matmul(
                        warm_ps[:, 0, :], warm_w[:], warm_x[:], start=True, stop=True
                    )
                for _ in range(NPROBE):  # N=256 cost probes
                    nc.tensor.matmul(
                        warm_ps[:, 1, 0:256], warm_w[:], warm_x[:, 0:256],
                        start=True, stop=True,
                    )

                # weight comp APs per (t, tap): t0/t3 read the raw w tile
                def wc(t, tap):
                    if t == 0:
                        return w_sb[:, 0, tap, :]
                    if t == 1:
                        return wt12[:, 0, tap, :]
                    if t == 2:
                        return wt12[:, 1, tap, :]
                    return w_sb[:, 2, tap, :]

                def epilogue(c, ps):
                    jd, hh = c // 2, c % 2
                    m0, m1, m2, m3 = (ps[:, t, :] for t in range(4))
                    # PSUM has a single DVE read port: stage m2 in SBUF so
                    # every op below has at most one PSUM operand.
                    c2 = ep.tile([COUT, 512], dt.float32, name=f"c2_{c}", tag="c2")
                    nc.vector.tensor_copy(c2[:], m2)
                    # even output slice: m0 + 0.5*(m1+m2)
                    p = ep.tile([COUT, 512], dt.float32, name=f"p_{c}", tag="pq")
                    nc.vector.tensor_add(p[:], m1, c2[:])
                    s = ep.tile([COUT, 512], dt.float32, name=f"s_{c}", tag="s")
                    nc.vector.scalar_tensor_tensor(
                        out=s[:], in0=p[:], scalar=0.5, in1=m0,
                        op0=mybir.AluOpType.mult, op1=mybir.AluOpType.add,
                    )
                    # odd output slice: 0.5*(m1-m2) - m3
                    q = ep.tile([COUT, 512], dt.float32, name=f"q_{c}", tag="pq")
                    nc.vector.tensor_sub(q[:], m1, c2[:])
                    so = ep.tile([COUT, 512], dt.float32, name=f"so_{c}", tag="s")
                    nc.vector.scalar_tensor_tensor(
                        out=so[:], in0=q[:], scalar=0.5, in1=m3,
                        op0=mybir.AluOpType.mult, op1=mybir.AluOpType.subtract,
                    )
                    for par, pre in ((0, s), (1, so)):
                        u = ep.tile([COUT, 512], dt.float32, name=f"u{par}_{c}", tag="u")
                        nc.vector.tensor_scalar(
                            out=u[:], in0=pre[:],
                            scalar1=scal["dscale"][:], scalar2=scal["bias_s"][:],
                            op0=mybir.AluOpType.mult, op1=mybir.AluOpType.add,
                        )
                        v = ep.tile([COUT, 512], dt.float32, name=f"v{par}_{c}", tag="v")
                        nc.vector.scalar_tensor_tensor(
                            out=v[:], in0=u[:], scalar=ALPHA, in1=u[:],
                            op0=mybir.AluOpType.mult, op1=mybir.AluOpType.max,
                        )
                        oc = op.tile([COUT, 512], dt.float32, name=f"oc{par}_{c}", tag="oc")
                        nc.vector.tensor_scalar(
                            out=oc[:], in0=v[:], scalar1=-CLAMP, scalar2=CLAMP,
                            op0=mybir.AluOpType.max, op1=mybir.AluOpType.min,
                        )
                        # output chunk index = 2*d + hh, d = 2*jd + par
                        nc.sync.dma_start(y_d[:, 4 * jd + 2 * par + hh, :], oc[:])

                for c in range(NCHUNK):
                    jd, hh = c // 2, c % 2
                    if hh == 0:  # stream 2 raw slices per jd step
                        for p in (2 * jd + 6, 2 * jd + 7):
                            if p < PD:
                                dma_slice(p, nc.sync if p % 2 == 0 else nc.scalar)
                    ps = psp.tile([COUT, 4, 512], dt.float32, name=f"ps_{c}", tag="ps")
                    h0 = 16 * hh
                    for t in range(4):
                        for tap in range(9):
                            kh, kw = tap // 3, tap % 3
                            rhs = xtilde[:, t, jd, h0 + kh : h0 + kh + 16, kw : kw + 32]
                            nc.tensor.matmul(
                                ps[:, t, :], wc(t, tap), rhs,
                                start=(tap == 0), stop=(tap == 8),
                            )
                    if hh == 1 and jd + 2 < NJD:
                        emit_transform(jd + 2)
                    epilogue(c, ps)
    nc.compile()
    return nc


def _get_nc():
    if "nc" not in _CACHED:
        _CACHED["nc"] = _build_nc()
    return _CACHED["nc"]


def kernel(x: np.ndarray, weight: np.ndarray, bias: np.ndarray) -> np.ndarray:
    global LAST_RESULTS
    import ml_dtypes

    io = ml_dtypes.bfloat16

    x = np.asarray(x)
    weight = np.asarray(weight, dtype=np.float32)
    bias = np.asarray(bias, dtype=np.float32)

    # [cout, cin, kd, kh, kw] -> [cin, kd, (kh kw), cout]
    w_prep = np.ascontiguousarray(
        weight.transpose(1, 2, 3, 4, 0).reshape(CIN, K, 9, COUT).astype(io)
    )
    # [cout, cin, kd, kh, kw] -> [cout, (kd kh kw), cin]  (for the dcoef chain)
    wt_prep = np.ascontiguousarray(
        weight.reshape(COUT, CIN, 27).transpose(0, 2, 1).astype(io)
    )
    b_prep = np.ascontiguousarray(bias.reshape(COUT, 1))

    xio = x.astype(io)
    in_maps = []
    for i in range(B):
        xp = np.zeros((CIN, PD, HP, WP), dtype=io)
        xp[:, 1 : D + 1, 1 : H + 1, 1 : W + 1] = xio[i]
        in_maps.append({"xp": xp, "w": w_prep, "wt": wt_prep, "bias": b_prep})

    nc = _get_nc()
    trace = bool(int(os.environ.get("CONV_TRACE", "0")))
    res = run_bass_kernel_spmd(
        nc,
        in_maps,
        core_ids=list(range(B)),
        trace=trace,
    )
    LAST_RESULTS = res
    out = np.stack(
        [r["y"].reshape(COUT, D, H, W) for r in res.results], axis=0
    ).astype(np.float32)
    return out


# revision 23
# speedup vs baseline: 1.3180x; 1.0346x over previous
"""Trainium2 Bass kernel for nn_Conv3DNorm (modulated conv3d + demod + lrelu + clamp).

Reference math (styles == ones):
    dcoef[cout] = rsqrt(sum_{cin,kd,kh,kw} weight^2 + 1e-8)
    y = conv3d(x, weight * dcoef, pad=1)            # per-sample, stride 1
    y = leaky_relu(y + bias, 0.2) * sqrt(2)
    y = clip(y, -256, 256)

Sharding: data-parallel over batch. Core i processes sample i (B=8 == n_cores).

Algorithm: 1D Winograd F(2,3) along the DEPTH axis.  The direct method needs
27 matmuls per 512-position chunk (64 chunks, 1692 matmuls total after
boundary-tap skips) = 361 us of bf16 PE time at N=512 roofline (213.3 ns +
~2.5 ns issue).  Winograd-D computes output-slice PAIRS (2jd, 2jd+1) from 4
transformed components, eliminating the kd tap dimension:
    xt[0] = xp[2jd]   - xp[2jd+2]        (xp = D/H/W zero-padded input)
    xt[1] = xp[2jd+1] + xp[2jd+2]
    xt[2] = xp[2jd+2] - xp[2jd+1]
    xt[3] = xp[2jd+1] - xp[2jd+3]
    m[t]  = conv2d_{kh,kw}(wc[t], xt[t])   # 9 taps x 4 comps = 36 matmuls
    y[2jd]   = m[0] + 0.5*(m[1] + m[2])    # 1/2 from the G-transform is
    y[2jd+1] = 0.5*(m[1] - m[2]) - m[3]    # folded into the inverse
with weight comps wc = [w[kd=0], w0+w1+w2, w0-w1+w2, w[kd=2]] (the scaled-by-
half comps are stored UNSCALED to save precision and DVE ops).  36 matmuls
per TWO direct chunks vs 54 -> 1.5x less PE time (~254 us).  bf16-pipeline
rel err simulated at 3.45e-3 (gate 2e-2).

Design notes carried over from the direct-conv baseline (HW-trace-derived):
  - bf16 N=512 matmuls run at roofline; matmuls whose rhs base is 2-byte-
    misaligned (odd element, the kw=1 taps) pay +13 ns; a second shifted
    xtilde copy does not fit SBUF (xtilde is 148 KB/partition), so ~5 us of
    penalty is accepted.
  - x arrives HOST-PADDED in all three dims ([CIN,34,34,34] bf16) so every
    DMA is a contiguous line-rate slice transfer; D-transform runs on DVE
    (whole-slice contiguous adds, bf16 2x rate) from a 6-slice raw ring.
  - HAM warm-up: throwaway matmuls on memset tiles keep the PE ramping to
    2.4 GHz before the first real matmul's data lands.  Four extra N=256
    probe matmuls measure the short-matmul cost for a future F(4,3) variant.
  - dcoef is computed entirely on DVE from a [cout,27,cin]-transposed bf16
    weight copy (square, tree-reduce taps, free-dim reduce over cin, then
    bit-hack rsqrt + 2 Newton steps) -- no PE matmul, no PSUM bank, none of
    the baseline's deferred-epilogue ordering dance.
  - PSUM: each chunk accumulates m[0..3] in a [cout,4,512] tile = 4 banks;
    pool bufs=2 uses all 8 banks; the warm-up tile shares slot 0 (write-
    after-write with chunk 1, safe in the in-order PE queue).
  - epilogue per chunk is 10 DVE ops (inverse transform + scale/lrelu/clamp
    for the even and odd output slices); DVE total ~75 us vs PE ~254 us.
"""

import os
import sys

for _p in (
    "/root/.axon_site",
    "/root/.axon_site/_ro/trn_rl_repo",
    "/root/.axon_site/_ro/pypackages",
):
    if os.path.isdir(_p) and _p not in sys.path:
        sys.path.insert(0, _p)

import numpy as np

import concourse.bass as bass  # noqa: F401
import concourse.mybir as mybir
import concourse.tile as tile
from concourse import bacc
from concourse.bass_utils import run_bass_kernel_spmd

# Problem constants (hardcoded per contract).
B = 8
CIN = 128
COUT = 128
D = H = W = 32
K = 3
PD = D + 2   # 34 padded depth slices
HP = H + 2   # 34
WP = W + 2   # 34
NJD = 16     # depth output-slice pairs
NCHUNK = 32  # (jd, half-of-H) chunks; each yields 2x512 outputs
EPS = 1e-8
S1 = float(np.sqrt(2.0))  # ACT_GAIN * GAIN
CLAMP = 256.0
ALPHA = 0.2
NWARM = 12
NPROBE = 4   # N=256 probe matmuls (F(4,3) feasibility measurement)
RING = 5     # raw-slice ring depth

LAST_RESULTS = None  # BassKernelResults of the most recent run (for test.py)

_CACHED = {}


def _build_nc():
    dt = mybir.dt
    io_dt = dt.bfloat16

    nc = bacc.Bacc("TRN2")
    xp_d = nc.dram_tensor("xp", [CIN, PD, HP, WP], io_dt, kind="ExternalInput")
    w_d = nc.dram_tensor("w", [CIN, K, 9, COUT], io_dt, kind="ExternalInput")
    wt_d = nc.dram_tensor("wt", [COUT, 27, CIN], io_dt, kind="ExternalInput")
    b_d = nc.dram_tensor("bias", [COUT, 1], dt.float32, kind="ExternalInput")
    # bf16 output: halves out-DMA; host upcasts (adds ~1.7e-3 rel err, budget ok)
    y_d = nc.dram_tensor("y", [COUT, 2 * D, 512], io_dt, kind="ExternalOutput")

    with tile.TileContext(nc) as tc:
        with (
            tc.tile_pool(name="big", bufs=1) as big,
            tc.tile_pool(name="ring", bufs=RING) as ring,
            tc.tile_pool(name="small", bufs=1) as small,
            tc.tile_pool(name="epi", bufs=2) as ep,
            tc.tile_pool(name="oc", bufs=2) as op,
        ):
            xtilde = big.tile([CIN, 4, NJD, HP, WP], io_dt)
            w_sb = big.tile([CIN, K, 9, COUT], io_dt)
            wt12 = big.tile([CIN, 2, 9, COUT], io_dt)
            wT_sb = big.tile([COUT, 27, CIN], io_dt)
            bias_sb = small.tile([COUT, 1], dt.float32)

            # warm-up operands (memset, ready before any DMA lands)
            warm_w = small.tile([CIN, COUT], io_dt)
            nc.vector.memset(warm_w[:], 0.0)
            warm_x = small.tile([CIN, 512], io_dt)
            nc.vector.memset(warm_x[:], 0.0)

            # ---- upfront DMAs, ordered for the startup critical path ----
            raw = {}

            def dma_slice(p, eng):
                raw[p] = ring.tile([CIN, HP, WP], io_dt, name=f"p{p}", tag="p")
                eng.dma_start(raw[p][:], xp_d[:, p])

            nc.sync.dma_start(w_sb[:], w_d[:])
            nc.scalar.dma_start(bias_sb[:], b_d[:])
            for p in range(4):  # transform(0) inputs
                dma_slice(p, nc.sync if p % 2 == 0 else nc.scalar)
            dma_slice(4, nc.sync)
            dma_slice(5, nc.scalar)
            nc.scalar.dma_start(wT_sb[:], wt_d[:])

            # per-partition lrelu slope for the ACT engine (imm alpha is
            # ignored by HW -- measured: Lrelu w/ float alpha ran plain relu)
            alpha_sb = small.tile([COUT, 1], dt.float32)
            nc.vector.memset(alpha_sb[:], ALPHA)

            # ---- depth transform for one jd (4 whole-slice contiguous adds,
            # DVE bf16 2x rate; GpSimd measured 3x slower - unusable) ----
            def emit_transform(jd):
                r0, r1, r2, r3 = (raw[2 * jd + i] for i in range(4))
                nc.vector.tensor_sub(xtilde[:, 0, jd], r0[:], r2[:])
                nc.vector.tensor_add(xtilde[:, 1, jd], r1[:], r2[:])
                nc.vector.tensor_sub(xtilde[:, 2, jd], r2[:], r1[:])
                nc.vector.tensor_sub(xtilde[:, 3, jd], r1[:], r3[:])

            emit_transform(0)
            emit_transform(1)

            # ---- weight comps: wt12[0]=w0+w1+w2, wt12[1]=w0-w1+w2 (after the
            # jd0/jd1 transforms: chunk 0 runs its t=0/3 groups first, so wt12
            # is only needed ~18 matmuls in) ----
            wt_tmp = ep.tile([CIN, 9, COUT], io_dt, name="wt_tmp", tag="pq")
            nc.vector.tensor_add(wt_tmp[:], w_sb[:, 0], w_sb[:, 2])
            nc.vector.tensor_add(wt12[:, 0], wt_tmp[:], w_sb[:, 1])
            nc.vector.tensor_sub(wt12[:, 1], wt_tmp[:], w_sb[:, 1])

            # ---- dcoef: DVE-only (square, tap tree-reduce, cin reduce,
            # bit-hack rsqrt + 2 Newton steps; the ACT-engine Sqrt was found
            # to reorder badly in the baseline) ----
            scal = {}

            def emit_dcoef():
                sqp = small.tile([COUT, 9, CIN], io_dt)
                acc_a = small.tile([COUT, CIN], dt.float32)
                acc_b = small.tile([COUT, CIN], dt.float32)
                nc.vector.memset(acc_a[:], 0.0)
                accs = [acc_a, acc_b]
                for g in range(3):
                    nc.vector.tensor_mul(
                        sqp[:], wT_sb[:, 9 * g : 9 * (g + 1), :],
                        wT_sb[:, 9 * g : 9 * (g + 1), :],
                    )
                    n = 9
                    while n > 1:
                        k = n // 2
                        nc.vector.tensor_add(
                            sqp[:, 0:k, :], sqp[:, 0:k, :], sqp[:, n - k : n, :]
                        )
                        n -= k
                    nc.vector.tensor_add(
                        accs[(g + 1) % 2][:], accs[g % 2][:], sqp[:, 0, :]
                    )
                acc = accs[1][:]  # after g=2, result is in acc_b
                dsum = small.tile([COUT, 1], dt.float32)
                nc.vector.tensor_reduce(
                    dsum[:], acc, axis=mybir.AxisListType.X, op=mybir.AluOpType.add
                )
                xe = small.tile([COUT, 1], dt.float32)
                nc.vector.tensor_scalar(
                    out=xe[:], in0=dsum[:], scalar1=float(EPS), scalar2=None,
                    op0=mybir.AluOpType.add,
                )
                r = small.tile([COUT, 1], dt.float32)
                nc.vector.tensor_scalar(
                    out=r[:].bitcast(dt.int32),
                    in0=xe[:].bitcast(dt.int32),
                    scalar1=1,
                    scalar2=None,
                    op0=mybir.AluOpType.logical_shift_right,
                )
                nc.vector.tensor_scalar(
                    out=r[:].bitcast(dt.int32),
                    in0=r[:].bitcast(dt.int32),
                    scalar1=-1,
                    scalar2=0x5F3759DF,
                    op0=mybir.AluOpType.mult,
                    op1=mybir.AluOpType.add,
                )
                t1 = small.tile([COUT, 1], dt.float32)
                t2 = small.tile([COUT, 1], dt.float32)
                for _ in range(2):  # r <- r * (1.5 - 0.5 * x * r^2)
                    nc.vector.tensor_mul(t1[:], r[:], r[:])
                    nc.vector.tensor_mul(t2[:], t1[:], xe[:])
                    nc.vector.tensor_scalar(
                        out=t2[:], in0=t2[:], scalar1=-0.5, scalar2=1.5,
                        op0=mybir.AluOpType.mult, op1=mybir.AluOpType.add,
                    )
                    nc.vector.tensor_mul(r[:], r[:], t2[:])
                dscale = small.tile([COUT, 1], dt.float32)
                nc.vector.tensor_scalar_mul(dscale[:], r[:], S1)
                bias_s = small.tile([COUT, 1], dt.float32)
                nc.vector.tensor_scalar_mul(bias_s[:], bias_sb[:], S1)
                scal["dscale"] = dscale
                scal["bias_s"] = bias_s

            emit_dcoef()

            # ---- main loop ----
            with tc.tile_pool(name="ps", bufs=2, space="PSUM") as psp:
                warm_ps = psp.tile([COUT, 4, 512], dt.float32, name="warm", tag="ps")
                for _ in range(NWARM):
                    nc.tensor.matmul(
                        warm_ps[:, 0, :], warm_w[:], warm_x[:], start=True, stop=True
                    )
                for _ in range(NPROBE):  # N=256 cost probes
                    nc.tensor.matmul(
                        warm_ps[:, 1, 0:256], warm_w[:], warm_x[:, 0:256],
                        start=True, stop=True,
                    )

                # weight comp APs per (t, tap): t0/t3 read the raw w tile
                def wc(t, tap):
                    if t == 0:
                        return w_sb[:, 0, tap, :]
                    if t == 1:
                        return wt12[:, 0, tap, :]
                    if t == 2:
                        return wt12[:, 1, tap, :]
                    return w_sb[:, 2, tap, :]

                def epilogue(c, ps):
                    jd, hh = c // 2, c % 2
                    m0, m1, m2, m3 = (ps[:, t, :] for t in range(4))
                    # PSUM has a single DVE read port: stage m2 in SBUF so
                    # every op below has at most one PSUM operand.
                    c2 = ep.tile([COUT, 512], dt.float32, name=f"c2_{c}", tag="c2")
                    nc.vector.tensor_copy(c2[:], m2)
                    # s2[:,0] = even slice pre-act = m0 + 0.5*(m1+m2)
                    # s2[:,1] = odd  slice pre-act = 0.5*(m1-m2) - m3
                    p = ep.tile([COUT, 512], dt.float32, name=f"p_{c}", tag="pq")
                    nc.vector.tensor_add(p[:], m1, c2[:])
                    s2 = ep.tile([COUT, 2, 512], dt.float32, name=f"s_{c}", tag="s")
                    nc.vector.scalar_tensor_tensor(
                        out=s2[:, 0, :], in0=p[:], scalar=0.5, in1=m0,
                        op0=mybir.AluOpType.mult, op1=mybir.AluOpType.add,
                    )
                    q = ep.tile([COUT, 512], dt.float32, name=f"q_{c}", tag="pq")
                    nc.vector.tensor_sub(q[:], m1, c2[:])
                    nc.vector.scalar_tensor_tensor(
                        out=s2[:, 1, :], in0=q[:], scalar=0.5, in1=m3,
                        op0=mybir.AluOpType.mult, op1=mybir.AluOpType.subtract,
                    )
                    # ACT: v2 = lrelu(s2*dscale + bias_s), both slices in one
                    # op.  Prelu with an AP alpha is the one variant the HW
                    # honors (Lrelu, and any float-imm alpha, runs plain relu).
                    v2 = ep.tile([COUT, 2, 512], dt.float32, name=f"v_{c}", tag="v")
                    nc.scalar.activation(
                        out=v2[:], in_=s2[:],
                        func=mybir.ActivationFunctionType.Prelu,
                        bias=scal["bias_s"][:], scale=scal["dscale"][:],
                        alpha=alpha_sb[:],
                    )
                    # DVE: clamp both slices in one single-src op, bf16 out
                    oc2 = op.tile([COUT, 2, 512], io_dt, name=f"oc_{c}", tag="oc")
                    nc.vector.tensor_scalar(
                        out=oc2[:], in0=v2[:], scalar1=-CLAMP, scalar2=CLAMP,
                        op0=mybir.AluOpType.max, op1=mybir.AluOpType.min,
                    )
                    # output chunk index = 2*d + hh, d = 2*jd + par
                    nc.sync.dma_start(y_d[:, 4 * jd + hh, :], oc2[:, 0, :])
                    nc.sync.dma_start(y_d[:, 4 * jd + 2 + hh, :], oc2[:, 1, :])

                for c in range(NCHUNK):
                    jd, hh = c // 2, c % 2
                    if hh == 0:  # stream 2 raw slices per jd step
                        for p in (2 * jd + 6, 2 * jd + 7):
                            if p < PD:
                                dma_slice(p, nc.sync if p % 2 == 0 else nc.scalar)
                    ps = psp.tile([COUT, 4, 512], dt.float32, name=f"ps_{c}", tag="ps")
                    h0 = 16 * hh
                    for t in (0, 3, 1, 2):  # raw-weight comps first (wt12 latency)
                        for tap in range(9):
                            kh, kw = tap // 3, tap % 3
                            rhs = xtilde[:, t, jd, h0 + kh : h0 + kh + 16, kw : kw + 32]
                            nc.tensor.matmul(
                                ps[:, t, :], wc(t, tap), rhs,
                                start=(tap == 0), stop=(tap == 8),
                            )
                    if hh == 1 and jd + 2 < NJD:
                        emit_transform(jd + 2)
                    epilogue(c, ps)
    nc.compile()
    return nc


def _get_nc():
    if "nc" not in _CACHED:
        _CACHED["nc"] = _build_nc()
    return _CACHED["nc"]


def kernel(x: np.ndarray, weight: np.ndarray, bias: np.ndarray) -> np.ndarray:
    global LAST_RESULTS
    import ml_dtypes

    io = ml_dtypes.bfloat16

    x = np.asarray(x)
    weight = np.asarray(weight, dtype=np.float32)
    bias = np.asarray(bias, dtype=np.float32)

    # [cout, cin, kd, kh, kw] -> [cin, kd, (kh kw), cout]
    w_prep = np.ascontiguousarray(
        weight.transpose(1, 2, 3, 4, 0).reshape(CIN, K, 9, COUT).astype(io)
    )
    # [cout, cin, kd, kh, kw] -> [cout, (kd kh kw), cin]  (for the dcoef chain)
    wt_prep = np.ascontiguousarray(
        weight.reshape(COUT, CIN, 27).transpose(0, 2, 1).astype(io)
    )
    b_prep = np.ascontiguousarray(bias.reshape(COUT, 1))

    xio = x.astype(io)
    in_maps = []
    for i in range(B):
        xp = np.zeros((CIN, PD, HP, WP), dtype=io)
        xp[:, 1 : D + 1, 1 : H + 1, 1 : W + 1] = xio[i]
        in_maps.append({"xp": xp, "w": w_prep, "wt": wt_prep, "bias": b_prep})

    nc = _get_nc()
    trace = bool(int(os.environ.get("CONV_TRACE", "0")))
    res = run_bass_kernel_spmd(
        nc,
        in_maps,
        core_ids=list(range(B)),
        trace=trace,
    )
    LAST_RESULTS = res
    out = np.stack(
        [r["y"].reshape(COUT, D, H, W) for r in res.results], axis=0
    ).astype(np.float32)
    return out


# revision 27
# speedup vs baseline: 1.3468x; 1.0218x over previous
"""Trainium2 Bass kernel for nn_Conv3DNorm (modulated conv3d + demod + lrelu + clamp).

Reference math (styles == ones):
    dcoef[cout] = rsqrt(sum_{cin,kd,kh,kw} weight^2 + 1e-8)
    y = conv3d(x, weight * dcoef, pad=1)            # per-sample, stride 1
    y = leaky_relu(y + bias, 0.2) * sqrt(2)
    y = clip(y, -256, 256)

Sharding: data-parallel over batch. Core i processes sample i (B=8 == n_cores).

Algorithm: 1D Winograd F(2,3) along the DEPTH axis.  The direct method needs
27 matmuls per 512-position chunk (64 chunks, 1692 matmuls total after
boundary-tap skips) = 361 us of bf16 PE time at N=512 roofline (213.3 ns +
~2.5 ns issue).  Winograd-D computes output-slice PAIRS (2jd, 2jd+1) from 4
transformed components, eliminating the kd tap dimension:
    xt[0] = xp[2jd]   - xp[2jd+2]        (xp = D/H/W zero-padded input)
    xt[1] = xp[2jd+1] + xp[2jd+2]
    xt[2] = xp[2jd+2] - xp[2jd+1]
    xt[3] = xp[2jd+1] - xp[2jd+3]
    m[t]  = conv2d_{kh,kw}(wc[t], xt[t])   # 9 taps x 4 comps = 36 matmuls
    y[2jd]   = m[0] + 0.5*(m[1] + m[2])    # 1/2 from the G-transform is
    y[2jd+1] = 0.5*(m[1] - m[2]) - m[3]    # folded into the inverse
with weight comps wc = [w[kd=0], w0+w1+w2, w0-w1+w2, w[kd=2]] (the scaled-by-
half comps are stored UNSCALED to save precision and DVE ops).  36 matmuls
per TWO direct chunks vs 54 -> 1.5x less PE time (~254 us).  bf16-pipeline
rel err simulated at 3.45e-3 (gate 2e-2).

Design notes carried over from the direct-conv baseline (HW-trace-derived):
  - bf16 N=512 matmuls run at roofline; matmuls whose rhs base is 2-byte-
    misaligned (odd element, the kw=1 taps) pay +13 ns; a second shifted
    xtilde copy does not fit SBUF (xtilde is 148 KB/partition), so ~5 us of
    penalty is accepted.
  - x arrives HOST-PADDED in all three dims ([CIN,34,34,34] bf16) so every
    DMA is a contiguous line-rate slice transfer; D-transform runs on DVE
    (whole-slice contiguous adds, bf16 2x rate) from a 6-slice raw ring.
  - HAM warm-up: throwaway matmuls on memset tiles keep the PE ramping to
    2.4 GHz before the first real matmul's data lands.  Four extra N=256
    probe matmuls measure the short-matmul cost for a future F(4,3) variant.
  - dcoef is computed entirely on DVE from a [cout,27,cin]-transposed bf16
    weight copy (square, tree-reduce taps, free-dim reduce over cin, then
    bit-hack rsqrt + 2 Newton steps) -- no PE matmul, no PSUM bank, none of
    the baseline's deferred-epilogue ordering dance.
  - PSUM: each chunk accumulates m[0..3] in a [cout,4,512] tile = 4 banks;
    pool bufs=2 uses all 8 banks; the warm-up tile shares slot 0 (write-
    after-write with chunk 1, safe in the in-order PE queue).
  - epilogue per chunk is 10 DVE ops (inverse transform + scale/lrelu/clamp
    for the even and odd output slices); DVE total ~75 us vs PE ~254 us.
"""

import os
import sys

for _p in (
    "/root/.axon_site",
    "/root/.axon_site/_ro/trn_rl_repo",
    "/root/.axon_site/_ro/pypackages",
):
    if os.path.isdir(_p) and _p not in sys.path:
        sys.path.insert(0, _p)

import numpy as np

import concourse.bass as bass  # noqa: F401
import concourse.mybir as mybir
import concourse.tile as tile
from concourse import bacc
from concourse.bass_utils import run_bass_kernel_spmd

# Problem constants (hardcoded per contract).
B = 8
CIN = 128
COUT = 128
D = H = W = 32
K = 3
PD = D + 2   # 34 padded depth slices
HP = H + 2   # 34
WP = W + 2   # 34
NJD = 16     # depth output-slice pairs
NCHUNK = 32  # (jd, half-of-H) chunks; each yields 2x512 outputs
EPS = 1e-8
S1 = float(np.sqrt(2.0))  # ACT_GAIN * GAIN
CLAMP = 256.0
ALPHA = 0.2
NWARM = 6
NPROBE = 4   # N=256 probe matmuls (F(4,3) feasibility measurement)
RING = 5     # raw-slice ring depth

LAST_RESULTS = None  # BassKernelResults of the most recent run (for test.py)

_CACHED = {}


def _build_nc():
    dt = mybir.dt
    io_dt = dt.bfloat16

    nc = bacc.Bacc("TRN2")
    xp_d = nc.dram_tensor("xp", [CIN, PD, HP, WP], io_dt, kind="ExternalInput")
    w_d = nc.dram_tensor("w", [CIN, K, 9, COUT], io_dt, kind="ExternalInput")
    wt_d = nc.dram_tensor("wt", [COUT, 27, CIN], io_dt, kind="ExternalInput")
    b_d = nc.dram_tensor("bias", [COUT, 1], dt.float32, kind="ExternalInput")
    # bf16 output: halves out-DMA; host upcasts (adds ~1.7e-3 rel err, budget ok)
    y_d = nc.dram_tensor("y", [COUT, 2 * D, 512], io_dt, kind="ExternalOutput")

    with tile.TileContext(nc) as tc:
        with (
            tc.tile_pool(name="big", bufs=1) as big,
            tc.tile_pool(name="ring", bufs=RING) as ring,
            tc.tile_pool(name="small", bufs=1) as small,
            tc.tile_pool(name="epi", bufs=2) as ep,
            tc.tile_pool(name="oc", bufs=2) as op,
        ):
            xtilde = big.tile([CIN, 4, NJD, HP, WP], io_dt)
            w_sb = big.tile([CIN, K, 9, COUT], io_dt)
            wt12 = big.tile([CIN, 2, 9, COUT], io_dt)
            wT_sb = big.tile([COUT, 27, CIN], io_dt)
            bias_sb = small.tile([COUT, 1], dt.float32)

            # warm-up operands (memset, ready before any DMA lands)
            warm_w = small.tile([CIN, COUT], io_dt)
            nc.vector.memset(warm_w[:], 0.0)
            warm_x = small.tile([CIN, 512], io_dt)
            nc.vector.memset(warm_x[:], 0.0)

            # ---- upfront DMAs, ordered for the startup critical path ----
            raw = {}

            def dma_slice(p, eng):
                raw[p] = ring.tile([CIN, HP, WP], io_dt, name=f"p{p}", tag="p")
                eng.dma_start(raw[p][:], xp_d[:, p])

            # chunk 0 needs w[:,0] + p0..p3 first; the rest of w follows in
            # consumption order (t=3 uses w[:,2]; wt12 needs all three)
            nc.sync.dma_start(w_sb[:, 0], w_d[:, 0])
            for p in range(4):  # transform(0) inputs
                dma_slice(p, nc.sync if p % 2 == 0 else nc.scalar)
            nc.sync.dma_start(w_sb[:, 2], w_d[:, 2])
            nc.scalar.dma_start(w_sb[:, 1], w_d[:, 1])
            dma_slice(4, nc.sync)
            dma_slice(5, nc.scalar)
            nc.scalar.dma_start(bias_sb[:], b_d[:])
            nc.scalar.dma_start(wT_sb[:], wt_d[:])

            # per-partition lrelu slope for the ACT engine (imm alpha is
            # ignored by HW -- measured: Lrelu w/ float alpha ran plain relu)
            alpha_sb = small.tile([COUT, 1], dt.float32)
            nc.vector.memset(alpha_sb[:], ALPHA)

            # ---- depth transform for one jd (4 whole-slice contiguous adds,
            # DVE bf16 2x rate; GpSimd measured 3x slower - unusable) ----
            def emit_transform(jd):
                r0, r1, r2, r3 = (raw[2 * jd + i] for i in range(4))
                nc.vector.tensor_sub(xtilde[:, 0, jd], r0[:], r2[:])
                nc.vector.tensor_add(xtilde[:, 1, jd], r1[:], r2[:])
                nc.vector.tensor_sub(xtilde[:, 2, jd], r2[:], r1[:])
                nc.vector.tensor_sub(xtilde[:, 3, jd], r1[:], r3[:])

            emit_transform(0)
            emit_transform(1)

            # ---- weight comps: wt12[0]=w0+w1+w2, wt12[1]=w0-w1+w2 (after the
            # jd0/jd1 transforms: chunk 0 runs its t=0/3 groups first, so wt12
            # is only needed ~18 matmuls in) ----
            wt_tmp = ep.tile([CIN, 9, COUT], io_dt, name="wt_tmp", tag="pq")
            nc.vector.tensor_add(wt_tmp[:], w_sb[:, 0], w_sb[:, 2])
            nc.vector.tensor_add(wt12[:, 0], wt_tmp[:], w_sb[:, 1])
            nc.vector.tensor_sub(wt12[:, 1], wt_tmp[:], w_sb[:, 1])

            # ---- dcoef: DVE-only (square, tap tree-reduce, cin reduce,
            # bit-hack rsqrt + 2 Newton steps; the ACT-engine Sqrt was found
            # to reorder badly in the baseline) ----
            scal = {}

            def emit_dcoef():
                # square + reduce in two halves (scratch fits SBUF; ~5 DVE ops)
                sq_sc = small.tile([COUT, 14, CIN], io_dt)
                tap_s = small.tile([COUT, 27], dt.float32)
                for a, b in ((0, 14), (14, 27)):
                    k = b - a
                    nc.vector.tensor_mul(
                        sq_sc[:, 0:k, :], wT_sb[:, a:b, :], wT_sb[:, a:b, :]
                    )
                    nc.vector.tensor_reduce(
                        tap_s[:, a:b], sq_sc[:, 0:k, :],
                        axis=mybir.AxisListType.X, op=mybir.AluOpType.add,
                    )
                dsum = small.tile([COUT, 1], dt.float32)
                nc.vector.tensor_reduce(
                    dsum[:], tap_s[:], axis=mybir.AxisListType.X,
                    op=mybir.AluOpType.add,
                )
                xe = small.tile([COUT, 1], dt.float32)
                nc.vector.tensor_scalar(
                    out=xe[:], in0=dsum[:], scalar1=float(EPS), scalar2=None,
                    op0=mybir.AluOpType.add,
                )
                r = small.tile([COUT, 1], dt.float32)
                nc.vector.tensor_scalar(
                    out=r[:].bitcast(dt.int32),
                    in0=xe[:].bitcast(dt.int32),
                    scalar1=1,
                    scalar2=None,
                    op0=mybir.AluOpType.logical_shift_right,
                )
                nc.vector.tensor_scalar(
                    out=r[:].bitcast(dt.int32),
                    in0=r[:].bitcast(dt.int32),
                    scalar1=-1,
                    scalar2=0x5F3759DF,
                    op0=mybir.AluOpType.mult,
                    op1=mybir.AluOpType.add,
                )
                t1 = small.tile([COUT, 1], dt.float32)
                t2 = small.tile([COUT, 1], dt.float32)
                for _ in range(2):  # r <- r * (1.5 - 0.5 * x * r^2)
                    nc.vector.tensor_mul(t1[:], r[:], r[:])
                    nc.vector.tensor_mul(t2[:], t1[:], xe[:])
                    nc.vector.tensor_scalar(
                        out=t2[:], in0=t2[:], scalar1=-0.5, scalar2=1.5,
                        op0=mybir.AluOpType.mult, op1=mybir.AluOpType.add,
                    )
                    nc.vector.tensor_mul(r[:], r[:], t2[:])
                dscale = small.tile([COUT, 1], dt.float32)
                nc.vector.tensor_scalar_mul(dscale[:], r[:], S1)
                bias_s = small.tile([COUT, 1], dt.float32)
                nc.vector.tensor_scalar_mul(bias_s[:], bias_sb[:], S1)
                scal["dscale"] = dscale
                scal["bias_s"] = bias_s

            emit_dcoef()

            # ---- main loop ----
            with tc.tile_pool(name="ps", bufs=2, space="PSUM") as psp:
                warm_ps = psp.tile([COUT, 4, 512], dt.float32, name="warm", tag="ps")
                for _ in range(NWARM):
                    nc.tensor.matmul(
                        warm_ps[:, 0, :], warm_w[:], warm_x[:], start=True, stop=True
                    )
                for _ in range(NPROBE):  # N=256 cost probes
                    nc.tensor.matmul(
                        warm_ps[:, 1, 0:256], warm_w[:], warm_x[:, 0:256],
                        start=True, stop=True,
                    )

                # weight comp APs per (t, tap): t0/t3 read the raw w tile
                def wc(t, tap):
                    if t == 0:
                        return w_sb[:, 0, tap, :]
                    if t == 1:
                        return wt12[:, 0, tap, :]
                    if t == 2:
                        return wt12[:, 1, tap, :]
                    return w_sb[:, 2, tap, :]

                def epilogue(c, ps):
                    jd, hh = c // 2, c % 2
                    m0, m1, m2, m3 = (ps[:, t, :] for t in range(4))
                    # PSUM has a single DVE read port: stage m2 in SBUF so
                    # every op below has at most one PSUM operand.
                    c2 = ep.tile([COUT, 512], dt.float32, name=f"c2_{c}", tag="c2")
                    nc.vector.tensor_copy(c2[:], m2)
                    # s2[:,0] = even slice pre-act = m0 + 0.5*(m1+m2)
                    # s2[:,1] = odd  slice pre-act = 0.5*(m1-m2) - m3
                    p = ep.tile([COUT, 512], dt.float32, name=f"p_{c}", tag="pq")
                    nc.vector.tensor_add(p[:], m1, c2[:])
                    s2 = ep.tile([COUT, 2, 512], dt.float32, name=f"s_{c}", tag="s")
                    nc.vector.scalar_tensor_tensor(
                        out=s2[:, 0, :], in0=p[:], scalar=0.5, in1=m0,
                        op0=mybir.AluOpType.mult, op1=mybir.AluOpType.add,
                    )
                    q = ep.tile([COUT, 512], dt.float32, name=f"q_{c}", tag="pq")
                    nc.vector.tensor_sub(q[:], m1, c2[:])
                    nc.vector.scalar_tensor_tensor(
                        out=s2[:, 1, :], in0=q[:], scalar=0.5, in1=m3,
                        op0=mybir.AluOpType.mult, op1=mybir.AluOpType.subtract,
                    )
                    # ACT: v2 = lrelu(s2*dscale + bias_s).  Prelu with an AP
                    # alpha is the one variant the HW honors (Lrelu, and any
                    # float-imm alpha, runs plain relu).  The last chunk runs
                    # per-slice so its first store overlaps the second slice's
                    # compute (tail latency).
                    v2 = ep.tile([COUT, 2, 512], dt.float32, name=f"v_{c}", tag="v")
                    oc2 = op.tile([COUT, 2, 512], io_dt, name=f"oc_{c}", tag="oc")
                    halves = ((slice(None),) if c < NCHUNK - 1 else (0, 1))
                    for hs in halves:
                        nc.scalar.activation(
                            out=v2[:, hs, :], in_=s2[:, hs, :],
                            func=mybir.ActivationFunctionType.Prelu,
                            bias=scal["bias_s"][:], scale=scal["dscale"][:],
                            alpha=alpha_sb[:],
                        )
                        # DVE: clamp (single-src 2x mode), bf16 out
                        nc.vector.tensor_scalar(
                            out=oc2[:, hs, :], in0=v2[:, hs, :],
                            scalar1=-CLAMP, scalar2=CLAMP,
                            op0=mybir.AluOpType.max, op1=mybir.AluOpType.min,
                        )
                        # output chunk index = 2*d + hh, d = 2*jd + par
                        if hs in (0, slice(None)):
                            nc.sync.dma_start(y_d[:, 4 * jd + hh, :], oc2[:, 0, :])
                        if hs in (1, slice(None)):
                            nc.sync.dma_start(y_d[:, 4 * jd + 2 + hh, :], oc2[:, 1, :])

                for c in range(NCHUNK):
                    jd, hh = c // 2, c % 2
                    if hh == 0:  # stream 2 raw slices per jd step
                        for p in (2 * jd + 6, 2 * jd + 7):
                            if p < PD:
                                dma_slice(p, nc.sync if p % 2 == 0 else nc.scalar)
                    ps = psp.tile([COUT, 4, 512], dt.float32, name=f"ps_{c}", tag="ps")
                    h0 = 16 * hh
                    for t in (0, 3, 1, 2):  # raw-weight comps first (wt12 latency)
                        for tap in range(9):
                            kh, kw = tap // 3, tap % 3
                            rhs = xtilde[:, t, jd, h0 + kh : h0 + kh + 16, kw : kw + 32]
                            nc.tensor.matmul(
                                ps[:, t, :], wc(t, tap), rhs,
                                start=(tap == 0), stop=(tap == 8),
                            )
                    if hh == 1 and jd + 2 < NJD:
                        emit_transform(jd + 2)
                    epilogue(c, ps)
    nc.compile()
    return nc


def _get_nc():
    if "nc" not in _CACHED:
        _CACHED["nc"] = _build_nc()
    return _CACHED["nc"]


def kernel(x: np.ndarray, weight: np.ndarray, bias: np.ndarray) -> np.ndarray:
    global LAST_RESULTS
    import ml_dtypes

    io = ml_dtypes.bfloat16

    x = np.asarray(x)
    weight = np.asarray(weight, dtype=np.float32)
    bias = np.asarray(bias, dtype=np.float32)

    # [cout, cin, kd, kh, kw] -> [cin, kd, (kh kw), cout]
    w_prep = np.ascontiguousarray(
        weight.transpose(1, 2, 3, 4, 0).reshape(CIN, K, 9, COUT).astype(io)
    )
    # [cout, cin, kd, kh, kw] -> [cout, (kd kh kw), cin]  (for the dcoef chain)
    wt_prep = np.ascontiguousarray(
        weight.reshape(COUT, CIN, 27).transpose(0, 2, 1).astype(io)
    )
    b_prep = np.ascontiguousarray(bias.reshape(COUT, 1))

    xio = x.astype(io)
    in_maps = []
    for i in range(B):
        xp = np.zeros((CIN, PD, HP, WP), dtype=io)
        xp[:, 1 : D + 1, 1 : H + 1, 1 : W + 1] = xio[i]
        in_maps.append({"xp": xp, "w": w_prep, "wt": wt_prep, "bias": b_prep})

    nc = _get_nc()
    trace = bool(int(os.environ.get("CONV_TRACE", "0")))
    res = run_bass_kernel_spmd(
        nc,
        in_maps,
        core_ids=list(range(B)),
        trace=trace,
    )
    LAST_RESULTS = res
    out = np.stack(
        [r["y"].reshape(COUT, D, H, W) for r in res.results], axis=0
    ).astype(np.float32)
    return out


# revision 31
# speedup vs baseline: 1.3704x; 1.0176x over previous
"""Trainium2 Bass kernel for nn_Conv3DNorm (modulated conv3d + demod + lrelu + clamp).

Reference math (styles == ones):
    dcoef[cout] = rsqrt(sum_{cin,kd,kh,kw} weight^2 + 1e-8)
    y = conv3d(x, weight * dcoef, pad=1)            # per-sample, stride 1
    y = leaky_relu(y + bias, 0.2) * sqrt(2)
    y = clip(y, -256, 256)

Sharding: data-parallel over batch. Core i processes sample i (B=8 == n_cores).

Algorithm: 1D Winograd F(2,3) along the DEPTH axis.  The direct method needs
27 matmuls per 512-position chunk (64 chunks, 1692 matmuls total after
boundary-tap skips) = 361 us of bf16 PE time at N=512 roofline (213.3 ns +
~2.5 ns issue).  Winograd-D computes output-slice PAIRS (2jd, 2jd+1) from 4
transformed components, eliminating the kd tap dimension:
    xt[0] = xp[2jd]   - xp[2jd+2]        (xp = D/H/W zero-padded input)
    xt[1] = xp[2jd+1] + xp[2jd+2]
    xt[2] = xp[2jd+2] - xp[2jd+1]
    xt[3] = xp[2jd+1] - xp[2jd+3]
    m[t]  = conv2d_{kh,kw}(wc[t], xt[t])   # 9 taps x 4 comps = 36 matmuls
    y[2jd]   = m[0] + 0.5*(m[1] + m[2])    # 1/2 from the G-transform is
    y[2jd+1] = 0.5*(m[1] - m[2]) - m[3]    # folded into the inverse
with weight comps wc = [w[kd=0], w0+w1+w2, w0-w1+w2, w[kd=2]] (the scaled-by-
half comps are stored UNSCALED to save precision and DVE ops).  36 matmuls
per TWO direct chunks vs 54 -> 1.5x less PE time (~254 us).  bf16-pipeline
rel err simulated at 3.45e-3 (gate 2e-2).

Design notes carried over from the direct-conv baseline (HW-trace-derived):
  - bf16 N=512 matmuls run at roofline; matmuls whose rhs base is 2-byte-
    misaligned (odd element, the kw=1 taps) pay +13 ns; a second shifted
    xtilde copy does not fit SBUF (xtilde is 148 KB/partition), so ~5 us of
    penalty is accepted.
  - x arrives HOST-PADDED in all three dims ([CIN,34,34,34] bf16) so every
    DMA is a contiguous line-rate slice transfer; D-transform runs on DVE
    (whole-slice contiguous adds, bf16 2x rate) from a 6-slice raw ring.
  - HAM warm-up: throwaway matmuls on memset tiles keep the PE ramping to
    2.4 GHz before the first real matmul's data lands.  Four extra N=256
    probe matmuls measure the short-matmul cost for a future F(4,3) variant.
  - dcoef is computed entirely on DVE from a [cout,27,cin]-transposed bf16
    weight copy (square, tree-reduce taps, free-dim reduce over cin, then
    bit-hack rsqrt + 2 Newton steps) -- no PE matmul, no PSUM bank, none of
    the baseline's deferred-epilogue ordering dance.
  - PSUM: each chunk accumulates m[0..3] in a [cout,4,512] tile = 4 banks;
    pool bufs=2 uses all 8 banks; the warm-up tile shares slot 0 (write-
    after-write with chunk 1, safe in the in-order PE queue).
  - epilogue per chunk is 10 DVE ops (inverse transform + scale/lrelu/clamp
    for the even and odd output slices); DVE total ~75 us vs PE ~254 us.
"""

import os
import sys

for _p in (
    "/root/.axon_site",
    "/root/.axon_site/_ro/trn_rl_repo",
    "/root/.axon_site/_ro/pypackages",
):
    if os.path.isdir(_p) and _p not in sys.path:
        sys.path.insert(0, _p)

import numpy as np

import concourse.bass as bass  # noqa: F401
import concourse.mybir as mybir
import concourse.tile as tile
from concourse import bacc
from concourse.bass_utils import run_bass_kernel_spmd

# Problem constants (hardcoded per contract).
B = 8
CIN = 128
COUT = 128
D = H = W = 32
K = 3
PD = D + 2   # 34 padded depth slices
HP = H + 2   # 34
WP = W + 2   # 34
NJD = 16     # depth output-slice pairs
NCHUNK = 32  # (jd, half-of-H) chunks; each yields 2x512 outputs
EPS = 1e-8
S1 = float(np.sqrt(2.0))  # ACT_GAIN * GAIN
CLAMP = 256.0
ALPHA = 0.2
NWARM = 10
NPROBE = 4   # N=256 probe matmuls (F(4,3) feasibility measurement)
RING = 5     # raw-slice ring depth

LAST_RESULTS = None  # BassKernelResults of the most recent run (for test.py)

_CACHED = {}


def _build_nc():
    dt = mybir.dt
    io_dt = dt.bfloat16

    nc = bacc.Bacc("TRN2")
    xp_d = nc.dram_tensor("xp", [CIN, PD, HP, WP], io_dt, kind="ExternalInput")
    w_d = nc.dram_tensor("w", [CIN, K, 9, COUT], io_dt, kind="ExternalInput")
    wt_d = nc.dram_tensor("wt", [COUT, 27, CIN], io_dt, kind="ExternalInput")
    b_d = nc.dram_tensor("bias", [COUT, 1], dt.float32, kind="ExternalInput")
    # bf16 output: halves out-DMA; host upcasts (adds ~1.7e-3 rel err, budget ok)
    y_d = nc.dram_tensor("y", [COUT, 2 * D, 512], io_dt, kind="ExternalOutput")

    with tile.TileContext(nc) as tc:
        with (
            tc.tile_pool(name="big", bufs=1) as big,
            tc.tile_pool(name="ring", bufs=RING) as ring,
            tc.tile_pool(name="small", bufs=1) as small,
            tc.tile_pool(name="epi", bufs=2) as ep,
            tc.tile_pool(name="oc", bufs=2) as op,
        ):
            xtilde = big.tile([CIN, 4, NJD, HP, WP], io_dt)
            w_sb = big.tile([CIN, K, 9, COUT], io_dt)
            wt12 = big.tile([CIN, 2, 9, COUT], io_dt)
            wT_sb = big.tile([COUT, 27, CIN], io_dt)
            bias_sb = small.tile([COUT, 1], dt.float32)

            # warm-up operands (memset, ready before any DMA lands)
            warm_w = small.tile([CIN, COUT], io_dt)
            nc.vector.memset(warm_w[:], 0.0)
            warm_x = small.tile([CIN, 512], io_dt)
            nc.vector.memset(warm_x[:], 0.0)

            # ---- upfront DMAs, ordered for the startup critical path ----
            raw = {}

            def dma_slice(p, eng):
                raw[p] = ring.tile([CIN, HP, WP], io_dt, name=f"p{p}", tag="p")
                eng.dma_start(raw[p][:], xp_d[:, p])

            # Queue order tuned for chunk-0 start: t0-comp needs p0,p2 (sync),
            # t3-comp needs p1,p3 (scalar) -- the first two mm groups are
            # gated by independent queues.  wT rides sync early so the dcoef
            # chain (which sits ahead of epilogue(c0) in the DVE queue)
            # doesn't delay the first PSUM release.
            dma_slice(0, nc.sync)
            dma_slice(1, nc.scalar)
            dma_slice(2, nc.sync)
            dma_slice(3, nc.scalar)
            nc.sync.dma_start(w_sb[:, 0], w_d[:, 0])
            nc.scalar.dma_start(w_sb[:, 2], w_d[:, 2])
            nc.scalar.dma_start(w_sb[:, 1], w_d[:, 1])
            nc.sync.dma_start(wT_sb[:], wt_d[:])
            dma_slice(4, nc.sync)
            dma_slice(5, nc.scalar)
            nc.scalar.dma_start(bias_sb[:], b_d[:])

            # per-partition lrelu slope for the ACT engine (imm alpha is
            # ignored by HW -- measured: Lrelu w/ float alpha ran plain relu)
            alpha_sb = small.tile([COUT, 1], dt.float32)
            nc.vector.memset(alpha_sb[:], ALPHA)

            # ---- depth transform for one jd (4 whole-slice contiguous adds,
            # DVE bf16 2x rate; GpSimd measured 3x slower - unusable) ----
            def emit_transform(jd):
                r0, r1, r2, r3 = (raw[2 * jd + i] for i in range(4))
                # comp order matches chunk-0 mm-group consumption (0,3,1,2)
                nc.vector.tensor_sub(xtilde[:, 0, jd], r0[:], r2[:])
                nc.vector.tensor_sub(xtilde[:, 3, jd], r1[:], r3[:])
                nc.vector.tensor_add(xtilde[:, 1, jd], r1[:], r2[:])
                nc.vector.tensor_sub(xtilde[:, 2, jd], r2[:], r1[:])

            emit_transform(0)
            emit_transform(1)

            # ---- weight comps: wt12[0]=w0+w1+w2, wt12[1]=w0-w1+w2 (after the
            # jd0/jd1 transforms: chunk 0 runs its t=0/3 groups first, so wt12
            # is only needed ~18 matmuls in) ----
            wt_tmp = ep.tile([CIN, 9, COUT], io_dt, name="wt_tmp", tag="pq")
            nc.vector.tensor_add(wt_tmp[:], w_sb[:, 0], w_sb[:, 2])
            nc.vector.tensor_add(wt12[:, 0], wt_tmp[:], w_sb[:, 1])
            nc.vector.tensor_sub(wt12[:, 1], wt_tmp[:], w_sb[:, 1])

            # ---- dcoef: DVE-only (square, tap tree-reduce, cin reduce,
            # bit-hack rsqrt + 2 Newton steps; the ACT-engine Sqrt was found
            # to reorder badly in the baseline) ----
            scal = {}

            def emit_dcoef():
                # square + reduce in two halves (scratch fits SBUF; ~5 DVE ops)
                sq_sc = small.tile([COUT, 14, CIN], io_dt)
                tap_s = small.tile([COUT, 27], dt.float32)
                for a, b in ((0, 14), (14, 27)):
                    k = b - a
                    nc.vector.tensor_mul(
                        sq_sc[:, 0:k, :], wT_sb[:, a:b, :], wT_sb[:, a:b, :]
                    )
                    nc.vector.tensor_reduce(
                        tap_s[:, a:b], sq_sc[:, 0:k, :],
                        axis=mybir.AxisListType.X, op=mybir.AluOpType.add,
                    )
                dsum = small.tile([COUT, 1], dt.float32)
                nc.vector.tensor_reduce(
                    dsum[:], tap_s[:], axis=mybir.AxisListType.X,
                    op=mybir.AluOpType.add,
                )
                xe = small.tile([COUT, 1], dt.float32)
                nc.vector.tensor_scalar(
                    out=xe[:], in0=dsum[:], scalar1=float(EPS), scalar2=None,
                    op0=mybir.AluOpType.add,
                )
                r = small.tile([COUT, 1], dt.float32)
                nc.vector.tensor_scalar(
                    out=r[:].bitcast(dt.int32),
                    in0=xe[:].bitcast(dt.int32),
                    scalar1=1,
                    scalar2=None,
                    op0=mybir.AluOpType.logical_shift_right,
                )
                nc.vector.tensor_scalar(
                    out=r[:].bitcast(dt.int32),
                    in0=r[:].bitcast(dt.int32),
                    scalar1=-1,
                    scalar2=0x5F3759DF,
                    op0=mybir.AluOpType.mult,
                    op1=mybir.AluOpType.add,
                )
                t1 = small.tile([COUT, 1], dt.float32)
                t2 = small.tile([COUT, 1], dt.float32)
                for _ in range(2):  # r <- r * (1.5 - 0.5 * x * r^2)
                    nc.vector.tensor_mul(t1[:], r[:], r[:])
                    nc.vector.tensor_mul(t2[:], t1[:], xe[:])
                    nc.vector.tensor_scalar(
                        out=t2[:], in0=t2[:], scalar1=-0.5, scalar2=1.5,
                        op0=mybir.AluOpType.mult, op1=mybir.AluOpType.add,
                    )
                    nc.vector.tensor_mul(r[:], r[:], t2[:])
                dscale = small.tile([COUT, 1], dt.float32)
                nc.vector.tensor_scalar_mul(dscale[:], r[:], S1)
                bias_s = small.tile([COUT, 1], dt.float32)
                nc.vector.tensor_scalar_mul(bias_s[:], bias_sb[:], S1)
                scal["dscale"] = dscale
                scal["bias_s"] = bias_s

            emit_dcoef()

            # ---- main loop ----
            with tc.tile_pool(name="ps", bufs=2, space="PSUM") as psp:
                warm_ps = psp.tile([COUT, 4, 512], dt.float32, name="warm", tag="ps")
                for _ in range(NWARM):
                    nc.tensor.matmul(
                        warm_ps[:, 0, :], warm_w[:], warm_x[:], start=True, stop=True
                    )
                for _ in range(NPROBE):  # N=256 cost probes
                    nc.tensor.matmul(
                        warm_ps[:, 1, 0:256], warm_w[:], warm_x[:, 0:256],
                        start=True, stop=True,
                    )

                # weight comp APs per (t, tap): t0/t3 read the raw w tile
                def wc(t, tap):
                    if t == 0:
                        return w_sb[:, 0, tap, :]
                    if t == 1:
                        return wt12[:, 0, tap, :]
                    if t == 2:
                        return wt12[:, 1, tap, :]
                    return w_sb[:, 2, tap, :]

                def epilogue(c, ps):
                    jd, hh = c // 2, c % 2
                    m0, m1, m2, m3 = (ps[:, t, :] for t in range(4))
                    # PSUM has a single DVE read port: stage m2 in SBUF so
                    # every op below has at most one PSUM operand.
                    c2 = ep.tile([COUT, 512], dt.float32, name=f"c2_{c}", tag="c2")
                    nc.vector.tensor_copy(c2[:], m2)
                    # s2[:,0] = even slice pre-act = m0 + 0.5*(m1+m2)
                    # s2[:,1] = odd  slice pre-act = 0.5*(m1-m2) - m3
                    p = ep.tile([COUT, 512], dt.float32, name=f"p_{c}", tag="pq")
                    nc.vector.tensor_add(p[:], m1, c2[:])
                    s2 = ep.tile([COUT, 2, 512], dt.float32, name=f"s_{c}", tag="s")
                    nc.vector.scalar_tensor_tensor(
                        out=s2[:, 0, :], in0=p[:], scalar=0.5, in1=m0,
                        op0=mybir.AluOpType.mult, op1=mybir.AluOpType.add,
                    )
                    q = ep.tile([COUT, 512], dt.float32, name=f"q_{c}", tag="pq")
                    nc.vector.tensor_sub(q[:], m1, c2[:])
                    nc.vector.scalar_tensor_tensor(
                        out=s2[:, 1, :], in0=q[:], scalar=0.5, in1=m3,
                        op0=mybir.AluOpType.mult, op1=mybir.AluOpType.subtract,
                    )
                    # ACT: v2 = lrelu(s2*dscale + bias_s).  Prelu with an AP
                    # alpha is the one variant the HW honors (Lrelu, and any
                    # float-imm alpha, runs plain relu).  The last chunk runs
                    # per-slice so its first store overlaps the second slice's
                    # compute (tail latency).
                    v2 = ep.tile([COUT, 2, 512], dt.float32, name=f"v_{c}", tag="v")
                    oc2 = op.tile([COUT, 2, 512], io_dt, name=f"oc_{c}", tag="oc")
                    halves = ((slice(None),) if c < NCHUNK - 1 else (0, 1))
                    for hs in halves:
                        nc.scalar.activation(
                            out=v2[:, hs, :], in_=s2[:, hs, :],
                            func=mybir.ActivationFunctionType.Prelu,
                            bias=scal["bias_s"][:], scale=scal["dscale"][:],
                            alpha=alpha_sb[:],
                        )
                        # DVE: clamp (single-src 2x mode), bf16 out
                        nc.vector.tensor_scalar(
                            out=oc2[:, hs, :], in0=v2[:, hs, :],
                            scalar1=-CLAMP, scalar2=CLAMP,
                            op0=mybir.AluOpType.max, op1=mybir.AluOpType.min,
                        )
                        # output chunk index = 2*d + hh, d = 2*jd + par
                        if hs in (0, slice(None)):
                            nc.sync.dma_start(y_d[:, 4 * jd + hh, :], oc2[:, 0, :])
                        if hs in (1, slice(None)):
                            nc.sync.dma_start(y_d[:, 4 * jd + 2 + hh, :], oc2[:, 1, :])

                for c in range(NCHUNK):
                    jd, hh = c // 2, c % 2
                    if hh == 0:  # stream 2 raw slices per jd step
                        for p in (2 * jd + 6, 2 * jd + 7):
                            if p < PD:
                                dma_slice(p, nc.sync if p % 2 == 0 else nc.scalar)
                    ps = psp.tile([COUT, 4, 512], dt.float32, name=f"ps_{c}", tag="ps")
                    h0 = 16 * hh
                    # chunk 0: raw-weight comps first (wt12 latency); later
                    # chunks: m1,m2,m0 early so the epilogue chain (c2 reads
                    # m2, then p/s/q) starts draining PSUM while the m3
                    # group is still on the PE.
                    t_order = (0, 3, 1, 2) if c == 0 else (1, 2, 0, 3)
                    for t in t_order:
                        for tap in range(9):
                            kh, kw = tap // 3, tap % 3
                            rhs = xtilde[:, t, jd, h0 + kh : h0 + kh + 16, kw : kw + 32]
                            nc.tensor.matmul(
                                ps[:, t, :], wc(t, tap), rhs,
                                start=(tap == 0), stop=(tap == 8),
                            )
                    if hh == 1 and jd + 2 < NJD:
                        emit_transform(jd + 2)
                    epilogue(c, ps)
    nc.compile()
    return nc


def _get_nc():
    if "nc" not in _CACHED:
        _CACHED["nc"] = _build_nc()
    return _CACHED["nc"]


def kernel(x: np.ndarray, weight: np.ndarray, bias: np.ndarray) -> np.ndarray:
    global LAST_RESULTS
    import ml_dtypes

    io = ml_dtypes.bfloat16

    x = np.asarray(x)
    weight = np.asarray(weight, dtype=np.float32)
    bias = np.asarray(bias, dtype=np.float32)

    # [cout, cin, kd, kh, kw] -> [cin, kd, (kh kw), cout]
    w_prep = np.ascontiguousarray(
        weight.transpose(1, 2, 3, 4, 0).reshape(CIN, K, 9, COUT).astype(io)
    )
    # [cout, cin, kd, kh, kw] -> [cout, (kd kh kw), cin]  (for the dcoef chain)
    wt_prep = np.ascontiguousarray(
        weight.reshape(COUT, CIN, 27).transpose(0, 2, 1).astype(io)
    )
    b_prep = np.ascontiguousarray(bias.reshape(COUT, 1))

    xio = x.astype(io)
    in_maps = []
    for i in range(B):
        xp = np.zeros((CIN, PD, HP, WP), dtype=io)
        xp[:, 1 : D + 1, 1 : H + 1, 1 : W + 1] = xio[i]
        in_maps.append({"xp": xp, "w": w_prep, "wt": wt_prep, "bias": b_prep})

    nc = _get_nc()
    trace = bool(int(os.environ.get("CONV_TRACE", "0")))
    res = run_bass_kernel_spmd(
        nc,
        in_maps,
        core_ids=list(range(B)),
        trace=trace,
    )
    LAST_RESULTS = res
    out = np.stack(
        [r["y"].reshape(COUT, D, H, W) for r in res.results], axis=0
    ).astype(np.float32)
    return out
